# revision 20
# baseline (speedup 1.0000x reference)
"""Trainium2 Bass kernel for ChunkTriangleAttentionStartingNode.

Computation (B=1, N=384, D=128, h=4, c=32):
  Z = LayerNorm(Z_raw) * ln_w + ln_b                     (over d_pair)
  bias[h,q,k]   = (Z @ W_b)[q,k,h]        (triangle bias, row-indexed by q)
  q,k,v         = split(Z @ W_qkv)        per pair-row i, heads h, dim c
  logits[i,h,q,k] = q.k / sqrt(c) + mask_bias[i,k] + bias[h,q,k]
  out = Z_raw + (sigmoid(Z@W_gate + gb) * softmax(logits) @ v) @ W_o + out_bias

Sharding: rows (first pair axis) split across 8 cores, 48 rows each; each
core computes its bias shard, AllGather produces the full [h,N,N] bias.

v4 rework (from the 689us baseline, trace-driven):
  - Phase 1 split: stats pass (DVE reduces + ACT square) in row groups,
    ONE batched sqrt per group (the scheduler freely hoists exp/tanh-set
    activations, so any per-row sqrt thrashes ACT tables), then the
    normalize+transpose pass overlapping the next group's stats.
  - Bias AllGather chunked in 3, issued as each third of the shard is
    written, so most of it hides under phase-1's tail.
  - Triangle bias applied by PE identity-matmul PSUM accumulation onto the
    QK logits (replaces the exp(bias) DVE multiply — the old DVE hotspot),
    with all 4 heads' logits in one 4-bank PSUM tile so the softmax exp is
    a single ACT op per (row, chunk).
  - Pre/post software pipeline with LAG rows: projections/casts run ahead,
    filling the AllGather window; projections own a separate 2-bank PSUM
    tag so they never sit inside the logits-tag rotation.
  - out_bias folded into the output-projection PSUM via a rank-1 matmul.
  - QK(kc+1) issued before wa/sum(kc) so the PE never head-of-line blocks
    on the exp of the current chunk.
"""

import os
import sys

os.environ.setdefault("NEURON_RT_RESET_CORES", "1")

for _p in ("/opt/trn_rl_repo",):
    if _p not in sys.path:
        sys.path.append(_p)

import numpy as np
import ml_dtypes

import concourse.bass as bass
import concourse.bacc as bacc
import concourse.tile as tile
from concourse import mybir

F32 = mybir.dt.float32
BF16 = mybir.dt.bfloat16
AF = mybir.ActivationFunctionType
ALU = mybir.AluOpType
AX = mybir.AxisListType

P = 128          # partitions
D = 128          # d_pair
NH = 4           # heads
CH = 32          # head dim
HC = NH * CH     # 128

LAG = int(os.environ.get("K_LAG", "6"))


def build_nc(N=384, n_cores=8):
    C3 = N // P           # chunks along the attention axis
    R = N // n_cores      # rows per core
    lag = min(LAG, R - 1)

    nc = bacc.Bacc(
        "TRN2",
        target_bir_lowering=False,
        debug=False,
        enable_asserts=False,
        num_devices=n_cores,
    )

    Zr = nc.dram_tensor("z_raw", [R, N, D], F32, kind="ExternalInput").ap()
    Zm = nc.dram_tensor("z_mask", [R, N], F32, kind="ExternalInput").ap()
    lnw_d = nc.dram_tensor("ln_w", [D], F32, kind="ExternalInput").ap()
    lnb_d = nc.dram_tensor("ln_b", [D], F32, kind="ExternalInput").ap()
    wb_d = nc.dram_tensor("w_b", [D, NH], F32, kind="ExternalInput").ap()
    wqkv_d = nc.dram_tensor("w_qkv", [D, 3 * HC], F32, kind="ExternalInput").ap()
    wg_d = nc.dram_tensor("w_gate", [D, HC], F32, kind="ExternalInput").ap()
    gb_d = nc.dram_tensor("gating_bias", [HC], F32, kind="ExternalInput").ap()
    wo_d = nc.dram_tensor("w_o", [HC, D], F32, kind="ExternalInput").ap()
    ob_d = nc.dram_tensor("out_bias", [D], F32, kind="ExternalInput").ap()
    OUT = nc.dram_tensor("out", [R, N, D], F32, kind="ExternalOutput").ap()

    id_bf_d = nc.inline_tensor(np.eye(P, dtype=ml_dtypes.bfloat16), "id_bf_c").ap()
    ones_d = nc.inline_tensor(
        np.full((P, CH), 2.0, dtype=ml_dtypes.bfloat16), "ones_c"
    ).ap()

    with tile.TileContext(nc) as tc:
        with (
            tc.tile_pool(name="const", bufs=1) as constp,
            tc.tile_pool(name="res", bufs=1) as resp,
            tc.tile_pool(name="work", bufs=3) as work,
            tc.tile_pool(name="stat", bufs=4) as statp,
            tc.tile_pool(name="wpool", bufs=4) as wpool,
            tc.tile_pool(name="pre", bufs=lag + 2) as prep,
            tc.tile_pool(name="ps", bufs=1, space="PSUM") as psum,
            tc.tile_pool(name="dram", bufs=1, space="DRAM") as dramp,
        ):
            # ---- constants / weights ----
            id_bf = constp.tile([P, P], BF16)
            nc.sync.dma_start(id_bf, id_bf_d)
            ones_bf = constp.tile([P, CH], BF16)
            nc.sync.dma_start(ones_bf, ones_d)

            lnw = constp.tile([D, 1], F32)
            nc.sync.dma_start(lnw, lnw_d[:, None])
            lnb = constp.tile([D, 1], F32)
            nc.sync.dma_start(lnb, lnb_d[:, None])
            gb = constp.tile([HC, 1], F32)
            nc.sync.dma_start(gb, gb_d[:, None])
            ngb = constp.tile([HC, 1], F32)
            nc.scalar.mul(ngb, gb, 0.5)
            eps_c = constp.tile([P, 1], F32)
            nc.gpsimd.memset(eps_c, 1e-5)
            neg1e9_c = constp.tile([P, 1], F32)
            nc.gpsimd.memset(neg1e9_c, -1e9)

            # rank-1 out_bias accumulation operands
            ones1 = constp.tile([1, P], F32)
            nc.gpsimd.memset(ones1, 1.0)
            obr3 = constp.tile([1, C3, P], F32, tag="obr3")
            for c in range(C3):
                nc.sync.dma_start(obr3[:, c, :], ob_d[None, :])

            wtmp = constp.tile([D, 3 * HC], F32, tag="wtmp")
            nc.sync.dma_start(wtmp, wqkv_d)
            wq = constp.tile([D, HC], BF16)
            nc.scalar.activation(wq, wtmp[:, 0:HC], AF.Copy, scale=CH ** -0.5)
            wk = constp.tile([D, HC], BF16)
            nc.scalar.copy(wk, wtmp[:, HC:2 * HC])
            wv = constp.tile([D, HC], BF16)
            nc.scalar.copy(wv, wtmp[:, 2 * HC:3 * HC])

            wgt = constp.tile([D, HC], F32, tag="wgt")
            nc.sync.dma_start(wgt, wg_d)
            wg = constp.tile([D, HC], BF16)
            nc.scalar.copy(wg, wgt)
            wot = constp.tile([HC, D], F32, tag="wot")
            nc.sync.dma_start(wot, wo_d)
            wo = constp.tile([HC, D], BF16)
            nc.scalar.copy(wo, wot)
            wbt = constp.tile([D, NH], F32, tag="wbt")
            nc.sync.dma_start(wbt, wb_d)
            wb = constp.tile([D, NH], BF16)
            nc.scalar.copy(wb, wbt)

            # mask bias columns: mb[kc][k, i] = (Z_mask[i, k] - 1) * 1e9
            mb = []
            for kc in range(C3):
                mk = work.tile([P, R], F32, tag="mk")
                nc.sync.dma_start(
                    mk, Zm[:, kc * P:(kc + 1) * P].rearrange("r p -> p r")
                )
                mbt = resp.tile([P, R], F32, tag=f"mb{kc}", name=f"mb{kc}")
                nc.scalar.activation(mbt, mk, AF.Identity, scale=1e9, bias=neg1e9_c)
                mb.append(mbt)

            # DRAM bounce buffers for the bias AllGather (3 chunks, each its
            # own Shared tensor: one collective writer per tensor)
            b_shard = dramp.tile([R, NH, N], BF16, tag="bshard")
            ag_edges = [R // 3, (2 * R) // 3, R]
            ag_bounds = [(0 if i == 0 else ag_edges[i - 1], ag_edges[i])
                         for i in range(3)]
            b_fulls = [
                dramp.tile(
                    [n_cores, hi - lo, NH, N], BF16, tag=f"bfull{i}",
                    addr_space="Shared", name=f"bfull{i}",
                )
                for i, (lo, hi) in enumerate(ag_bounds)
            ]

            # ---- phase 1: LN stats (pass a) -> batched sqrt -> normalize +
            # transpose + bias shard (pass b), pipelined in row groups so a
            # group's pass b overlaps the next group's pass a.  Batched
            # sqrt (1 per group) keeps the scheduler from thrashing ACT
            # table sets between sqrt and the exp/tanh-set activations.
            s1_all = resp.tile([P, R, C3], F32, tag="s1_all")
            s2_all = resp.tile([P, R, C3], F32, tag="s2_all")
            rsig_all = resp.tile([P, R, C3], F32, tag="rsig_all")
            nmr_all = resp.tile([P, R, C3], F32, tag="nmr_all")
            Zt = resp.tile([P, R * C3 * P], BF16, tag="Zt")

            def stats_row(q):
                zrowA = work.tile([P, C3, P], F32, tag="zrowA")
                nc.sync.dma_start(zrowA, Zr[q].rearrange("(c p) d -> p c d", p=P))
                nc.vector.reduce_sum(s1_all[:, q, :], zrowA, axis=AX.X)
                zsq = work.tile([P, C3, P], F32, tag="zsq")
                nc.scalar.square(zsq, zrowA)
                nc.vector.reduce_sum(s2_all[:, q, :], zsq, axis=AX.X)

            def stats_finish(r0, r1):
                n = (r1 - r0) * C3
                s1g = s1_all[:, r0:r1, :].rearrange("p r c -> p (r c)")
                s2g = s2_all[:, r0:r1, :].rearrange("p r c -> p (r c)")
                rsg = rsig_all[:, r0:r1, :].rearrange("p r c -> p (r c)")
                nmg = nmr_all[:, r0:r1, :].rearrange("p r c -> p (r c)")
                mug = statp.tile([P, n], F32, tag="mug")
                nc.vector.tensor_scalar_mul(mug, s1g, 1.0 / D)
                musq = statp.tile([P, n], F32, tag="musq")
                nc.vector.tensor_mul(musq, mug, mug)
                varg = statp.tile([P, n], F32, tag="varg")
                nc.vector.scalar_tensor_tensor(
                    varg, s2g, 1.0 / D, musq, op0=ALU.mult, op1=ALU.subtract
                )
                stdg = statp.tile([P, n], F32, tag="stdg")
                nc.scalar.activation(stdg, varg, AF.Sqrt, bias=eps_c)
                nc.vector.reciprocal(rsg, stdg)
                nc.vector.scalar_tensor_tensor(
                    nmg, mug, -1.0, rsg, op0=ALU.mult, op1=ALU.mult
                )

            def norm_row(q):
                zrow = work.tile([P, C3, P], F32, tag="zrow")
                nc.sync.dma_start(zrow, Zr[q].rearrange("(c p) d -> p c d", p=P))
                tp = psum.tile([P, C3, P], BF16, tag="acc", bufs=1, name="tp")
                for c in range(C3):
                    zn = work.tile([P, P], BF16, tag="zn")
                    if c == 0:  # chunk 0 on ACT, rest on DVE (engine balance)
                        nc.scalar.activation(
                            zn, zrow[:, c, :], AF.Identity,
                            bias=nmr_all[:, q, c:c + 1],
                            scale=rsig_all[:, q, c:c + 1],
                        )
                    else:
                        nc.vector.tensor_scalar(
                            zn, zrow[:, c, :],
                            rsig_all[:, q, c:c + 1], nmr_all[:, q, c:c + 1],
                            op0=ALU.mult, op1=ALU.add,
                        )
                    nc.tensor.transpose(tp[:, c, :], zn, id_bf)
                nc.vector.tensor_scalar(
                    Zt[:, q * C3 * P:(q + 1) * C3 * P].rearrange(
                        "p (c q2) -> p c q2", c=C3
                    ),
                    tp, lnw, lnb, op0=ALU.mult, op1=ALU.add,
                )
                bp = psum.tile([NH, N], F32, tag="acc", bufs=1, name="bp")
                nc.tensor.matmul(bp, wb, Zt[:, q * C3 * P:(q + 1) * C3 * P])
                bsb = work.tile([NH, N], BF16, tag="bsb")
                nc.scalar.copy(bsb, bp)
                nc.sync.dma_start(b_shard[q], bsb)

            # the bias AllGather is split in 3 so the first two chunks run
            # under the tail of phase 1; each fires as soon as its rows'
            # shards are written
            def maybe_ag(q):
                if q + 1 in ag_edges:
                    idx = ag_edges.index(q + 1)
                    lo, hi = ag_bounds[idx]
                    nc.gpsimd.collective_compute(
                        "AllGather",
                        ALU.bypass,
                        replica_groups=[list(range(n_cores))],
                        ins=[b_shard[lo:hi].opt()],
                        outs=[b_fulls[idx].opt()],
                    )

            GROUP = (R + 1) // 2
            done_stats = 0
            for g0 in range(0, R, GROUP):
                g1 = min(g0 + GROUP, R)
                for q in range(g0, g1):
                    stats_row(q)
                stats_finish(g0, g1)
                # previous group's pass b (overlaps this group's pass a in
                # the scheduler since its rsig/nmr are already final)
                if g0 > 0:
                    for q in range(done_stats, g0):
                        norm_row(q)
                        maybe_ag(q)
                    done_stats = g0
            for q in range(done_stats, R):
                norm_row(q)
                maybe_ag(q)
            bt = []
            for c, (lo, hi) in enumerate(ag_bounds):
                npart = n_cores * (hi - lo)
                btq = resp.tile([npart, NH, N], BF16, tag=f"bt{c}", name=f"bt{c}")
                nc.sync.dma_start(
                    btq, b_fulls[c].rearrange("dev r h k -> (dev r) h k")
                )
                bt.append(btq)

            # ---------- pre/post row pipeline pieces ----------
            zrow2s = [None] * R
            qk_sbs = [None] * R
            vsbs = [None] * R
            ths = [None] * R

            def pre(i):
                zrow2 = prep.tile([P, C3, P], F32, tag="zrow2")
                nc.sync.dma_start(zrow2, Zr[i].rearrange("(c p) d -> p c d", p=P))
                zrow2s[i] = zrow2
                zt_row = Zt[:, i * C3 * P:(i + 1) * C3 * P]
                # two waves through one 2-bank proj tag so projections never
                # sit inside the logits-tag rotation (inter-row critical path)
                pjA = psum.tile([P, 2, 512], F32, tag="proj", bufs=1, name="pjA")
                nc.tensor.matmul(pjA[:, 0, 0:N], wq, zt_row)
                nc.tensor.matmul(pjA[:, 1, 0:N], wk, zt_row)
                qk_sb = prep.tile([P, 2, N], BF16, tag="qk_sb")
                nc.vector.tensor_copy(qk_sb, pjA[:, :, 0:N])
                qk_sbs[i] = qk_sb
                pjB = psum.tile([P, 2, 512], F32, tag="proj", bufs=1, name="pjB")
                nc.tensor.matmul(pjB[:, 1, 0:N], wg, zt_row)
                for c in range(C3):
                    nc.tensor.matmul(
                        pjB[:, 0, c * P:(c + 1) * P],
                        zt_row[:, c * P:(c + 1) * P],
                        wv,
                    )
                vsb = prep.tile([P, C3, P], BF16, tag="vsb")
                nc.vector.tensor_copy(
                    vsb, pjB[:, 0, 0:N].rearrange("p (c q2) -> p c q2", c=C3)
                )
                vsbs[i] = vsb
                th = prep.tile([P, N], BF16, tag="th")
                nc.scalar.activation(th, pjB[:, 1, 0:N], AF.Tanh, scale=0.5, bias=ngb)
                ths[i] = th

            def post(i):
                qt = qk_sbs[i][:, 0, :]
                kt = qk_sbs[i][:, 1, :]
                vsb = vsbs[i]
                wap3 = psum.tile([P, 2, 512], F32, tag="acc", bufs=1, name="wap3")
                wap = wap3[:, 0, 0:N]
                sp = wap3[:, 1, 0:N]

                wms = [None] * C3

                def qk_chunk(kc):
                    w_t4 = wpool.tile([P, NH, N], BF16, tag="wt")
                    lg = psum.tile([P, NH, 512], F32, tag="lg4", bufs=1, name="lg")
                    for h in range(NH):
                        nc.tensor.matmul(
                            lg[:, h, 0:N],
                            kt[CH * h:CH * (h + 1), kc * P:(kc + 1) * P],
                            qt[CH * h:CH * (h + 1), :],
                            tile_position=(CH * h, 0),
                            start=True, stop=False, skip_group_check=True,
                        )
                    # triangle bias added on the PE (identity matmul
                    # accumulate) instead of an exp(bias) multiply on DVE
                    for h in range(NH):
                        nc.tensor.matmul(
                            lg[:, h, 0:N], id_bf, Bb[kc][:, h, :],
                            start=False, stop=True, skip_group_check=True,
                        )
                    nc.scalar.activation(
                        w_t4, lg[:, :, 0:N], AF.Exp, bias=mb[kc][:, i:i + 1]
                    )
                    wms[kc] = w_t4

                def wa_chunk(kc):
                    wm4 = wms[kc]
                    for h in range(NH):
                        nc.tensor.matmul(
                            wap[CH * h:CH * (h + 1), :],
                            vsb[:, kc, CH * h:CH * (h + 1)],
                            wm4[:, h, :],
                            start=(kc == 0),
                            stop=(kc == C3 - 1),
                            skip_group_check=True,
                            tile_position=(0, CH * h),
                        )
                    for h in range(NH):
                        nc.tensor.matmul(
                            sp[CH * h:CH * (h + 1), :],
                            ones_bf,
                            wm4[:, h, :],
                            start=(kc == 0),
                            stop=(kc == C3 - 1),
                            skip_group_check=True,
                            tile_position=(0, CH * h),
                        )

                # stagger: QK(kc+1) issues before wa/sum(kc) so the PE stream
                # never stalls behind the exp/mul of the current chunk
                qk_chunk(0)
                for kc in range(1, C3):
                    qk_chunk(kc)
                    wa_chunk(kc - 1)
                wa_chunk(C3 - 1)

                rs = work.tile([P, N], F32, tag="rs")
                nc.vector.reciprocal_approx_fast(rs, sp)
                wan = work.tile([P, N], F32, tag="wan")
                nc.vector.tensor_mul(wan, wap, rs)
                gwa = work.tile([P, N], BF16, tag="gwa")
                nc.vector.scalar_tensor_tensor(
                    gwa, ths[i], 1.0, wan, op0=ALU.add, op1=ALU.mult
                )
                out_ps = psum.tile([P, C3, P], F32, tag="proj", bufs=1, name="out_ps")
                nc.tensor.matmul(
                    out_ps.rearrange("p c d -> p (c d)"),
                    ones1, obr3.rearrange("o c d -> o (c d)"),
                    start=True, stop=False, skip_group_check=True,
                )
                for c in range(C3):
                    nc.tensor.matmul(
                        out_ps[:, c, :], gwa[:, c * P:(c + 1) * P], wo,
                        start=False, stop=True, skip_group_check=True,
                    )
                fin = work.tile([P, C3, P], F32, tag="fin")
                nc.vector.tensor_add(fin, out_ps, zrow2s[i])
                nc.sync.dma_start(OUT[i].rearrange("(c p) d -> p c d", p=P), fin)
                zrow2s[i] = qk_sbs[i] = vsbs[i] = ths[i] = None

            # issue the first LAG rows' projections before the Eb section so
            # they run during the AllGather window
            for i in range(lag):
                pre(i)

            # transposed bias, resident per k-chunk: Bb[kc][k, h, q];
            # kc-major so Bb[0] (needed by the first post) completes first.
            # AG chunk c holds q-columns {dev*R + r, r in [lo,hi)} — a
            # strided slice of Bb's q axis.
            Bb = [
                resp.tile([P, NH, N], BF16, tag=f"bb{kc}", name=f"bb{kc}")
                for kc in range(C3)
            ]
            for kc in range(C3):
                for c, (lo, hi) in enumerate(ag_bounds):
                    npart = n_cores * (hi - lo)
                    for h in range(NH):
                        tp2 = psum.tile([P, P], BF16, tag="acc", bufs=1, name="tp2")
                        nc.tensor.transpose(
                            tp2[:, 0:npart],
                            bt[c][:, h, kc * P:(kc + 1) * P],
                            id_bf[0:npart, 0:npart],
                        )
                        nc.scalar.copy(
                            Bb[kc][:, h, :].rearrange(
                                "p (dev r) -> p dev r", dev=n_cores
                            )[:, :, lo:hi],
                            tp2[:, 0:npart].rearrange(
                                "p (dev r) -> p dev r", dev=n_cores
                            ),
                        )

            # ---- phase 2: per-row attention, software-pipelined ----
            for i in range(R):
                post(i)
                if i + lag < R:
                    pre(i + lag)

    nc.compile()
    return nc


_CACHE = {}


def get_nc(N=384, n_cores=8):
    key = (N, n_cores)
    if key not in _CACHE:
        _CACHE[key] = build_nc(N, n_cores)
    return _CACHE[key]


def make_in_maps(inputs, N=384, n_cores=8):
    R = N // n_cores
    Z = np.ascontiguousarray(np.asarray(inputs["Z_raw"], dtype=np.float32))
    M = np.ascontiguousarray(np.asarray(inputs["Z_mask"], dtype=np.float32))
    shared = {
        "ln_w": np.ascontiguousarray(np.asarray(inputs["ln_w"], np.float32)),
        "ln_b": np.ascontiguousarray(np.asarray(inputs["ln_b"], np.float32)),
        "w_b": np.ascontiguousarray(np.asarray(inputs["W_b"], np.float32)),
        "w_qkv": np.ascontiguousarray(np.asarray(inputs["W_qkv"], np.float32)),
        "w_gate": np.ascontiguousarray(np.asarray(inputs["W_gate"], np.float32)),
        "gating_bias": np.ascontiguousarray(
            np.asarray(inputs["gating_bias"], np.float32)
        ),
        "w_o": np.ascontiguousarray(np.asarray(inputs["W_o"], np.float32)),
        "out_bias": np.ascontiguousarray(np.asarray(inputs["out_bias"], np.float32)),
    }
    in_maps = []
    for c in range(n_cores):
        m = dict(shared)
        m["z_raw"] = np.ascontiguousarray(Z[0, c * R:(c + 1) * R])
        m["z_mask"] = np.ascontiguousarray(M[0, c * R:(c + 1) * R])
        in_maps.append(m)
    return in_maps


def kernel(**inputs):
    from concourse.bass_utils import run_bass_kernel_spmd

    N, n_cores = 384, 8
    nc = get_nc(N, n_cores)
    in_maps = make_in_maps(inputs, N, n_cores)
    res = run_bass_kernel_spmd(nc, in_maps, list(range(n_cores)))
    out = np.concatenate([res.results[c]["out"] for c in range(n_cores)], axis=0)
    return out.reshape(1, N, N, D).astype(np.float32)
